# revision 1
# baseline (speedup 1.0000x reference)
"""DiffAttn2d TRN2 Bass kernel.

Sharding: 8 cores = 2 (batch) x 4 (head-groups of 2 heads / 4 doubled-heads).
Per core, everything is computed channel-major (scores transposed: [key, query])
so softmax needs no transposes:
  - dots^T via row-packed K=16 fp32r matmuls (4 doubled heads in 4 PE row groups)
  - one ACT exp pass PSUM->SBUF (bf16), the only O(n^2) elementwise op
  - attn @ v as Z^T = [v | ones]^T @ exp^T (bf16, col-block packed 2 dheads/matmul);
    the ones column yields the softmax denominators for free
  - u = Z0/s0 - lam*Z1/s1 via reciprocal rows (exp(-ln(s))) broadcast with
    tiny selector matmuls, folded with a +/-1 matmul
  - LayerNorm stats via ones-matmuls; rsqrt via exp(-0.5*ln(var+eps))
  - sigmoid gating via exp/ln (same ACT table set as exp: zero table switches)
  - y^T = Wout^T @ gated, row-parallel partials summed on host (+ bout)
"""
import sys
sys.path.insert(0, "/opt/trn_rl_repo")

import math
import numpy as np
import ml_dtypes

import concourse.bass as bass
import concourse.bacc as bacc_mod
import concourse.mybir as mybir
from concourse.tile import TileContext
from concourse.bass_utils import run_bass_kernel_spmd

F = mybir.dt.float32
R = mybir.dt.float32r
BF = mybir.dt.bfloat16
AF = mybir.ActivationFunctionType
AL = mybir.AluOpType

H, DH = 8, 16
DEPTH = 1
LAMBDA_INIT = 0.8 - 0.6 * math.exp(-0.3 * DEPTH)
LN_EPS = 1e-5
B, N, DIM = 2, 2048, 256
NC = 8

_cached = {}


def build_kernel():
    nc = bacc_mod.Bacc()
    xT = nc.declare_dram_parameter("xT", [DIM, N], R, isOutput=False)
    wqp = nc.declare_dram_parameter("wqp", [DIM, 128], R, isOutput=False)
    wkp = nc.declare_dram_parameter("wkp", [DIM, 128], R, isOutput=False)
    wv = nc.declare_dram_parameter("wv", [DIM, 64], R, isOutput=False)
    wgc = nc.declare_dram_parameter("wgc", [DIM, 64], R, isOutput=False)
    wout = nc.declare_dram_parameter("wout", [32, 2, 256], R, isOutput=False)  # [ch, bank, out]
    sel = nc.declare_dram_parameter("sel", [4, 256], R, isOutput=False)        # [:, 0:128]=A, [:,128:]=B
    fold = nc.declare_dram_parameter("fold", [128, 32], R, isOutput=False)
    smu = nc.declare_dram_parameter("smu", [32, 4], R, isOutput=False)         # cols 0:2 mu-pass, 2:4 m2-pass
    ones32 = nc.declare_dram_parameter("ones32", [1, 32], R, isOutput=False)
    onescol = nc.declare_dram_parameter("onescol", [128, 32], BF, isOutput=False)
    gam = nc.declare_dram_parameter("gam", [32, 1], F, isOutput=False)
    bet = nc.declare_dram_parameter("bet", [32, 1], F, isOutput=False)
    nbg = nc.declare_dram_parameter("nbg", [32, 2], F, isOutput=False)
    epsc = nc.declare_dram_parameter("epsc", [1, 1], F, isOutput=False)
    yT = nc.declare_dram_parameter("yT", [DIM, N], F, isOutput=True)

    with TileContext(nc) as tc:
        with tc.tile_pool(name="pers", bufs=1) as pers, \
             tc.tile_pool(name="work", bufs=6) as work, \
             tc.tile_pool(name="epi", bufs=2) as epi, \
             tc.tile_pool(name="epi1", bufs=1) as epi1, \
             tc.tile_pool(name="zsp", bufs=4) as zsp, \
             tc.tile_pool(name="dp", bufs=2, space="PSUM") as dp, \
             tc.tile_pool(name="zp", bufs=2, space="PSUM") as zp:

            # ---------------- load persistent data ----------------
            # prewarm the ACT exp/ln table set during the DMA window (the
            # PSEUDO_LOAD_ACT_FUNC_SET fires before the first ACTIVATE)
            warm = pers.tile([1, 8], F, tag="warm")
            nc.vector.memset(warm[:], 0.0)
            nc.scalar.activation(warm[:], warm[:], AF.Exp)

            xt = pers.tile([128, 2, N], R, tag="xt")      # [:, f, :] feature chunk f
            xTr = xT.rearrange("(f p) n -> p f n", p=128)
            for f in range(2):
                nc.sync.dma_start(out=xt[:, f, :], in_=xTr[:, f, :])
            twkp = pers.tile([128, 2, 128], R, tag="twkp")
            nc.sync.dma_start(out=twkp[:], in_=wkp.rearrange("(f p) m -> p f m", p=128))
            twqp = pers.tile([128, 2, 128], R, tag="twqp")
            nc.sync.dma_start(out=twqp[:], in_=wqp.rearrange("(f p) m -> p f m", p=128))
            twv = pers.tile([128, 2, 64], R, tag="twv")
            nc.sync.dma_start(out=twv[:], in_=wv.rearrange("(f p) m -> p f m", p=128))
            tones = pers.tile([128, 32], BF, tag="tones")
            nc.sync.dma_start(out=tones[:], in_=onescol[:])
            twg = pers.tile([128, 2, 64], R, tag="twg")
            nc.sync.dma_start(out=twg[:], in_=wgc.rearrange("(f p) m -> p f m", p=128))
            twout = pers.tile([32, 2, 256], R, tag="twout")
            nc.sync.dma_start(out=twout[:], in_=wout[:])
            tsel = pers.tile([4, 256], R, tag="tsel")
            nc.sync.dma_start(out=tsel[:], in_=sel[:])
            tfold = pers.tile([128, 32], R, tag="tfold")
            nc.sync.dma_start(out=tfold[:], in_=fold[:])
            tsmu = pers.tile([32, 4], R, tag="tsmu")
            nc.sync.dma_start(out=tsmu[:], in_=smu[:])
            tones32 = pers.tile([1, 32], R, tag="tones32")
            nc.sync.dma_start(out=tones32[:], in_=ones32[:])
            tgam = pers.tile([32, 1], F, tag="tgam")
            nc.sync.dma_start(out=tgam[:], in_=gam[:])
            tbet = pers.tile([32, 1], F, tag="tbet")
            nc.sync.dma_start(out=tbet[:], in_=bet[:])
            tnbg = pers.tile([32, 2], F, tag="tnbg")
            nc.sync.dma_start(out=tnbg[:], in_=nbg[:])
            teps = pers.tile([1, 1], F, tag="teps")
            nc.sync.dma_start(out=teps[:], in_=epsc[:])

            # ---------------- projections ----------------
            # q^T / k^T packed: partition 32d+j (j<16) = channel j of dhead d
            qTp = pers.tile([128, N], R, tag="qTp")
            kTp = pers.tile([128, N], R, tag="kTp")
            vpp = pers.tile([128, 16, 128], BF, tag="vpp")

            def proj_qk(dst, w, it):
                ps = dp.tile([128, 1024], F, tag="dots")
                for f in range(2):
                    nc.tensor.matmul(ps[:, 0:512], w[:, f, :], xt[:, f, it * 512:(it + 1) * 512],
                                     start=(f == 0), stop=(f == 1))
                nc.vector.tensor_copy(dst[:, it * 512:(it + 1) * 512], ps[:, 0:512])

            def proj_v(jc):
                ps = dp.tile([128, 1024], F, tag="dots")
                for f in range(2):
                    nc.tensor.matmul(ps[:, 0:64], xt[:, f, jc * 128:(jc + 1) * 128], twv[:, f, :],
                                     start=(f == 0), stop=(f == 1))
                nc.vector.tensor_copy(vpp[:, jc, 0:32], ps[:, 0:32])
                nc.vector.tensor_copy(vpp[:, jc, 64:96], ps[:, 32:64])
                nc.vector.tensor_copy(vpp[:, jc, 32:64], tones[:])
                nc.vector.tensor_copy(vpp[:, jc, 96:128], tones[:])

            proj_qk(kTp, twkp, 0)
            proj_qk(qTp, twqp, 0)
            proj_qk(qTp, twqp, 1)
            for jc in range(4):
                proj_v(jc)
            for it in range(1, 4):
                proj_qk(kTp, twkp, it)
            proj_qk(qTp, twqp, 2)
            proj_qk(qTp, twqp, 3)
            for jc in range(4, 16):
                proj_v(jc)

            # ---------------- main: attention ----------------
            saved = {}

            def attention(ip, interleave=None):
                i0 = ip * 1024
                zA = zp.tile([128, 1024], F, tag="z")
                zB = zp.tile([128, 1024], F, tag="z")
                zbank = (zA, zA, zB, zB)
                for jc in range(16):
                    if interleave is not None and jc >= 7:
                        next(interleave, None)
                    j0 = jc * 128
                    for pair in range(2):
                        ebfs = []
                        dts = []
                        for dd in range(2):
                            d = 2 * pair + dd
                            r0 = 32 * d
                            dt_ = dp.tile([128, 1024], F, tag="dots")
                            for h in range(2):
                                nc.tensor.matmul(
                                    dt_[:, h * 512:(h + 1) * 512],
                                    kTp[r0:r0 + 16, j0:j0 + 128],
                                    qTp[r0:r0 + 16, i0 + h * 512:i0 + (h + 1) * 512],
                                    start=True, stop=True,
                                    tile_position=(r0, 0),
                                )
                            eb = work.tile([128, 1024], BF, tag="ebf")
                            nc.scalar.activation(eb[:], dt_[:], AF.Exp)
                            ebfs.append(eb)
                            dts.append(dt_)
                        for dd in range(2):
                            d = 2 * pair + dd
                            zt = zbank[d]
                            for h in range(2):
                                nc.tensor.matmul(
                                    zt[64 * dd:64 * dd + 64, h * 512:(h + 1) * 512],
                                    vpp[:, jc, 64 * pair:64 * pair + 64],
                                    ebfs[dd][:, h * 512:(h + 1) * 512],
                                    start=(jc == 0), stop=(jc == 15),
                                    tile_position=(0, 64 * dd),
                                    skip_group_check=True,
                                )

                # hand Z off to SBUF immediately so the Z psum slots free up
                zsA = zsp.tile([128, 1024], F, tag="zs")
                zsB = zsp.tile([128, 1024], F, tag="zs")
                nc.vector.tensor_copy(zsA[:], zA[:])
                nc.vector.tensor_copy(zsB[:], zB[:])
                srows = epi.tile([4, 1024], F, tag="srows")
                nc.sync.dma_start(out=srows[0:2, :], in_=zsA[32:128:64, :])
                nc.sync.dma_start(out=srows[2:4, :], in_=zsB[32:128:64, :])
                saved[ip] = (zsA, zsB, srows)

            def epilogue(ip):
                i0 = ip * 1024
                zsA, zsB, srows = saved[ip]
                rinv = epi1.tile([4, 1024], R, tag="rinv")
                nc.scalar.activation(srows[:], srows[:], AF.Ln)
                nc.scalar.activation(rinv[:], srows[:], AF.Exp, scale=-1.0)
                yield

                us = [epi1.tile([32, 1024], R, tag=f"us{b}", name=f"us{b}") for b in range(2)]
                sts = [epi1.tile([1, 1024], R, tag=f"sts{b}", name=f"sts{b}") for b in range(2)]
                sq = [epi1.tile([32, 1024], R, tag=f"sq{b}", name=f"sq{b}") for b in range(2)]
                bmus = [epi1.tile([32, 1024], F, tag=f"bmus{b}", name=f"bmus{b}") for b in range(2)]
                msq = [epi1.tile([1, 1024], F, tag=f"msq{b}", name=f"msq{b}") for b in range(2)]
                rs = [epi1.tile([1, 1024], R, tag=f"rs{b}", name=f"rs{b}") for b in range(2)]
                gt = [epi1.tile([32, 1024], R, tag=f"gt{b}", name=f"gt{b}") for b in range(2)]

                tts = []
                for b, zs in ((0, zsA), (1, zsB)):
                    bf_ = dp.tile([128, 1024], F, tag="dots")
                    for nt in range(2):
                        nc.tensor.matmul(bf_[:, nt * 512:(nt + 1) * 512],
                                         tsel[:, 128 * b:128 * (b + 1)],
                                         rinv[:, nt * 512:(nt + 1) * 512],
                                         start=True, stop=True)
                    tt = epi1.tile([128, 1024], R, tag=f"tt{b}", name=f"tt{b}")
                    nc.vector.tensor_tensor(tt[:], zs[:], bf_[:], AL.mult)
                    tts.append(tt)
                    yield
                for b in range(2):
                    uu = dp.tile([128, 1024], F, tag="dots")
                    for nt in range(2):
                        nc.tensor.matmul(uu[0:32, nt * 512:(nt + 1) * 512],
                                         tfold[:],
                                         tts[b][:, nt * 512:(nt + 1) * 512],
                                         start=True, stop=True)
                    nc.vector.tensor_copy(us[b][:], uu[0:32, :])
                    yield

                for b in range(2):
                    st = dp.tile([128, 1024], F, tag="dots")
                    for nt in range(2):
                        nc.tensor.matmul(st[0:1, nt * 512:(nt + 1) * 512],
                                         tsmu[:, 0:1],
                                         us[b][:, nt * 512:(nt + 1) * 512],
                                         start=True, stop=True)
                    nc.vector.tensor_copy(sts[b][:], st[0:1, :])
                    nc.vector.tensor_tensor(sq[b][:], us[b][:].bitcast(F), us[b][:].bitcast(F), AL.mult)
                    yield
                for b in range(2):
                    bm = dp.tile([128, 1024], F, tag="dots")
                    for nt in range(2):
                        nc.tensor.matmul(bm[0:32, nt * 512:(nt + 1) * 512], tones32[:],
                                         sts[b][:, nt * 512:(nt + 1) * 512],
                                         start=True, stop=True)
                    nc.vector.tensor_copy(bmus[b][:], bm[0:32, :])
                    nc.scalar.activation(msq[b][:], sts[b][:].bitcast(F), AF.Square)
                    yield
                for b in range(2):
                    st2 = dp.tile([128, 1024], F, tag="dots")
                    for nt in range(2):
                        nc.tensor.matmul(st2[0:1, nt * 512:(nt + 1) * 512],
                                         tsmu[:, 0:1],
                                         sq[b][:, nt * 512:(nt + 1) * 512],
                                         start=True, stop=True)
                    nc.vector.tensor_tensor(msq[b][:], st2[0:1, :], msq[b][:], AL.subtract)
                    nc.scalar.activation(msq[b][:], msq[b][:], AF.Ln, bias=teps[:])
                    nc.scalar.activation(rs[b][:], msq[b][:], AF.Exp, scale=-0.5)
                    yield
                for b in range(2):
                    brs = dp.tile([128, 1024], F, tag="dots")
                    for nt in range(2):
                        nc.tensor.matmul(brs[0:32, nt * 512:(nt + 1) * 512], tones32[:],
                                         rs[b][:, nt * 512:(nt + 1) * 512],
                                         start=True, stop=True)
                    t1 = epi1.tile([32, 1024], F, tag=f"t1{b}", name=f"t1{b}")
                    nc.vector.tensor_tensor(t1[:], us[b][:].bitcast(F), bmus[b][:], AL.subtract)
                    nc.vector.tensor_tensor(t1[:], t1[:], brs[0:32, :], AL.mult)
                    nc.vector.tensor_scalar(t1[:], t1[:], tgam[:], tbet[:], AL.mult, AL.add)
                    nc.vector.tensor_tensor(gt[b][:], t1[:],
                                            sgp[:, (2 * ip + b) * 1024:(2 * ip + b + 1) * 1024], AL.mult)
                    yield

                # output projection: yT[o, i] partials
                for oh in range(2):
                    yp = dp.tile([128, 1024], F, tag="dots")
                    for b in range(2):
                        for nt in range(2):
                            nc.tensor.matmul(yp[:, nt * 512:(nt + 1) * 512],
                                             twout[:, b, oh * 128:(oh + 1) * 128],
                                             gt[b][:, nt * 512:(nt + 1) * 512],
                                             start=(b == 0), stop=(b == 1))
                    ys = epi.tile([128, 1024], F, tag="ys")
                    nc.vector.tensor_copy(ys[:], yp[:])
                    nc.sync.dma_start(out=yT[oh * 128:(oh + 1) * 128, i0:i0 + 1024], in_=ys[:])
                    yield

            attention(0)
            # gates directly bank-packed: sgp [32, 4096], block (ip, b) at col
            # 1024*(2*ip+b).  sig = exp(-ln(exp(-(g+bg)) + 1))
            sgp = pers.tile([32, 4096], F, tag="sgp")
            for ip in range(2):
                for b in range(2):
                    ps = dp.tile([128, 1024], F, tag="dots")
                    for nt in range(2):
                        for f in range(2):
                            nc.tensor.matmul(
                                ps[0:32, nt * 512:(nt + 1) * 512],
                                twg[:, f, 32 * b:32 * b + 32],
                                xt[:, f, ip * 1024 + nt * 512:ip * 1024 + (nt + 1) * 512],
                                start=(f == 0), stop=(f == 1))
                    c0 = (2 * ip + b) * 1024
                    nc.scalar.activation(sgp[:, c0:c0 + 1024], ps[0:32, :], AF.Exp,
                                         scale=-1.0, bias=tnbg[:, b:b + 1])
            nc.scalar.activation(sgp[:], sgp[:], AF.Ln, bias=1.0)
            nc.scalar.activation(sgp[:], sgp[:], AF.Exp, scale=-1.0)

            gen0 = epilogue(0)
            attention(1, interleave=gen0)
            for _ in gen0:
                pass
            for _ in epilogue(1):
                pass

    nc.finalize()
    return nc


def _prep_core_inputs(inputs, bi, hg, lam):
    scale = DH ** -0.5
    x = np.asarray(inputs["x"], np.float32)
    Wq = np.asarray(inputs["Wq"], np.float32)
    Wkv = np.asarray(inputs["Wkv"], np.float32)
    Wout = np.asarray(inputs["Wout"], np.float32)
    Wg = np.asarray(inputs["Wg"], np.float32)
    bg = np.asarray(inputs["bg"], np.float32)
    g_ = np.asarray(inputs["ln_gamma"], np.float32)
    b_ = np.asarray(inputs["ln_beta"], np.float32)
    li = np.float32(1.0 - LAMBDA_INIT)

    c0 = 64 * hg
    wq_c = Wq[:, c0:c0 + 64] * scale
    wk_c = Wkv[:, c0:c0 + 64]
    wv_c = Wkv[:, 256 + c0:256 + c0 + 64]
    wg_c = Wg[:, c0:c0 + 64]
    wout_c = Wout[c0:c0 + 64, :]

    wqp = np.zeros((256, 128), np.float32)
    wkp = np.zeros((256, 128), np.float32)
    for d in range(4):
        wqp[:, 32 * d:32 * d + 16] = wq_c[:, 16 * d:16 * d + 16]
        wkp[:, 32 * d:32 * d + 16] = wk_c[:, 16 * d:16 * d + 16]

    woutp = np.zeros((32, 2, 256), np.float32)
    woutp[:, 0, :] = wout_c[0:32, :]
    woutp[:, 1, :] = wout_c[32:64, :]

    sel = np.zeros((4, 256), np.float32)
    sel[0, 0:32] = 1.0
    sel[1, 64:96] = lam
    sel[2, 128:160] = 1.0
    sel[3, 192:224] = lam

    fold = np.zeros((128, 32), np.float32)
    for r in range(32):
        fold[r, r] = 1.0
        fold[64 + r, r] = -1.0

    smu = np.zeros((32, 4), np.float32)
    smu[:, 0] = 1.0 / 32.0
    smu[:, 3] = 1.0 / 32.0

    ones32 = np.ones((1, 32), np.float32)
    onescol = np.zeros((128, 32), ml_dtypes.bfloat16)
    onescol[:, 0] = 1.0

    return {
        "xT": np.ascontiguousarray(x[bi].T),
        "wqp": wqp, "wkp": wkp,
        "wv": np.ascontiguousarray(wv_c),
        "wgc": np.ascontiguousarray(wg_c),
        "wout": woutp,
        "sel": sel, "fold": fold, "smu": smu,
        "ones32": ones32, "onescol": onescol,
        "epsc": np.full((1, 1), LN_EPS, np.float32),
        "gam": (g_[0:32] * li).reshape(32, 1).astype(np.float32),
        "bet": (b_[0:32] * li).reshape(32, 1).astype(np.float32),
        "nbg": (-bg[c0:c0 + 64]).reshape(64, 1).astype(np.float32),
    }


def kernel(**inputs) -> np.ndarray:
    lq1 = np.asarray(inputs["lq1"], np.float64)
    lk1 = np.asarray(inputs["lk1"], np.float64)
    lq2 = np.asarray(inputs["lq2"], np.float64)
    lk2 = np.asarray(inputs["lk2"], np.float64)
    lam = float(np.exp(np.sum(lq1 * lk1)) - np.exp(np.sum(lq2 * lk2)) + LAMBDA_INIT)
    bout = np.asarray(inputs["bout"], np.float32)

    if "nc" not in _cached:
        _cached["nc"] = build_kernel()
    nc = _cached["nc"]

    in_maps = []
    for c in range(NC):
        bi, hg = c // 4, c % 4
        in_maps.append(_prep_core_inputs(inputs, bi, hg, lam))

    import os
    trace = bool(int(os.environ.get("BASS_KERNEL_TRACE", "0")))
    res = run_bass_kernel_spmd(nc, in_maps, list(range(NC)), trace=trace)
    _cached["exec_time_ns"] = res.exec_time_ns
    _cached["trace"] = res.instructions_and_trace
    out = np.zeros((B, N, DIM), np.float32)
    for c in range(NC):
        bi = c // 4
        out[bi] += res.results[c]["yT"].T
    out += bout
    return out



# revision 21
# speedup vs baseline: 1.7325x; 1.7325x over previous
"""DiffAttn2d TRN2 Bass kernel (v2: query-major epilogue + ACT/Pool exp split).

Sharding: 8 cores = 2 (batch) x 4 (head-groups of 2 heads / 4 doubled-heads).
Per core (4 dheads d=0..3, heads m=d//2), n=2048 queries in 4 blocks of 512:
  - scores^T [keys, queries] via row-packed K=16 fp32r matmuls (baseline style)
  - exp split between ACT (direct psum->sbuf) and GPSIMD pow(e,x) (after a DVE
    psum->sbuf staging copy); both write bf16
  - attn@v TRANSPOSED: stationary = exp-scores slice [128k, 128q], moving =
    [v_m | 1] (33 cols) -> Z[q, (d,c)] in psum, softmax denominators free.
    Cost ~33 cols/slice instead of 512.
  - epilogue in query-major on DVE/Pool: reciprocal denominators, lambda fold,
    pair subtraction, LayerNorm stats via free-dim reduce, rsqrt via pow(x,-.5)
    on Pool, gating via tanh (sigma = .5 tanh(x/2)+.5, folded into gamma/beta)
  - gated -> bf16, DMA-transposed ([128q,128c'] -> [c',q]) and projected with a
    partition-duplicated Wout (K=64, tile rows 64k -> psum bank k)
  - yT partials summed on host (+ bout)
"""
import sys
sys.path.insert(0, "/opt/trn_rl_repo")

import math
import os
import numpy as np
import ml_dtypes

import concourse.bass as bass
import concourse.bacc as bacc_mod
import concourse.mybir as mybir
from concourse.tile import TileContext
from concourse.bass_utils import run_bass_kernel_spmd

F = mybir.dt.float32
R = mybir.dt.float32r
BF = mybir.dt.bfloat16
AF = mybir.ActivationFunctionType
AL = mybir.AluOpType
AX = mybir.AxisListType

H, DH = 8, 16
DEPTH = 1
LAMBDA_INIT = 0.8 - 0.6 * math.exp(-0.3 * DEPTH)
LN_EPS = 1e-5
B, N, DIM = 2, 2048, 256
NC = 8

NB = 4          # query blocks per core
NQ = 512        # queries per block
NTILE = NB * 16 * 2  # exp tiles total (b, jc, pair)
import os as _os
NPOOL = int(_os.environ.get("NPOOL", "50"))
EPI_LEVEL = int(_os.environ.get("EPI_LEVEL", "9"))
EPI_POOL = int(_os.environ.get("EPI_POOL", "1"))
DBG = int(_os.environ.get("KDBG", "0"))

_cached = {}


def _pool_set():
    if NPOOL <= 0:
        return set()
    return {int(round(j * NTILE / NPOOL)) for j in range(NPOOL)}


def build_kernel():
    nc = bacc_mod.Bacc()
    xT = nc.declare_dram_parameter("xT", [DIM, N], R, isOutput=False)
    wqp = nc.declare_dram_parameter("wqp", [DIM, 128], R, isOutput=False)
    wkp = nc.declare_dram_parameter("wkp", [DIM, 128], R, isOutput=False)
    wvg = nc.declare_dram_parameter("wvg", [DIM, 256], R, isOutput=False)
    bgrow = nc.declare_dram_parameter("bgrow", [1, 256], R, isOutput=False)
    onesrow = nc.declare_dram_parameter("onesrow", [1, 128], R, isOutput=False)
    wout2 = nc.declare_dram_parameter("wout2", [128, 2, 128], BF, isOutput=False)
    lamq = nc.declare_dram_parameter("lamq", [128, 16], F, isOutput=False)
    gamq = nc.declare_dram_parameter("gamq", [128, 256], F, isOutput=False)
    betq = nc.declare_dram_parameter("betq", [128, 256], F, isOutput=False)
    yT = nc.declare_dram_parameter("yT", [DIM, N], F, isOutput=True)
    dzs = nc.declare_dram_parameter("dzs", [NB, 128, 4, 132], F, isOutput=True)
    duu = nc.declare_dram_parameter("duu", [NB, 128, 4, 2, 32], F, isOutput=True)
    dgp = nc.declare_dram_parameter("dgp", [NB, 128, 2, 128], F, isOutput=True)
    dtg = nc.declare_dram_parameter("dtg", [128, 16, 64], F, isOutput=True)

    pool_set = _pool_set()

    with TileContext(nc) as tc:
        with tc.tile_pool(name="pers", bufs=1) as pers, \
             tc.tile_pool(name="ebp", bufs=8) as ebp, \
             tc.tile_pool(name="scp", bufs=6) as scp, \
             tc.tile_pool(name="epi", bufs=2) as epi, \
             tc.tile_pool(name="dp", bufs=3, space="PSUM") as dp, \
             tc.tile_pool(name="zp", bufs=1, space="PSUM") as zp:

            # prewarm the exp/tanh ACT table during the DMA window
            warm = pers.tile([1, 8], F, tag="warm")
            nc.vector.memset(warm[:], 0.0)
            nc.scalar.activation(warm[:], warm[:], AF.Exp)

            # ---------------- persistent loads ----------------
            xt = pers.tile([128, 2, N], R, tag="xt")
            xTr = xT.rearrange("(f p) n -> p f n", p=128)
            for f in range(2):
                nc.sync.dma_start(out=xt[:, f, 0:1024], in_=xTr[:, f, 0:1024])
            twkp = pers.tile([128, 2, 128], R, tag="twkp")
            nc.sync.dma_start(out=twkp[:], in_=wkp.rearrange("(f p) m -> p f m", p=128))
            twqp = pers.tile([128, 2, 128], R, tag="twqp")
            nc.sync.dma_start(out=twqp[:], in_=wqp.rearrange("(f p) m -> p f m", p=128))
            twvg = pers.tile([128, 2, 256], R, tag="twvg")
            nc.sync.dma_start(out=twvg[:], in_=wvg.rearrange("(f p) m -> p f m", p=128))
            tbg = pers.tile([1, 256], R, tag="tbg")
            nc.sync.dma_start(out=tbg[:], in_=bgrow[:])
            tones = pers.tile([1, 128], R, tag="tones")
            nc.sync.dma_start(out=tones[:], in_=onesrow[:])
            for f in range(2):
                nc.sync.dma_start(out=xt[:, f, 1024:2048], in_=xTr[:, f, 1024:2048])
            twout = pers.tile([128, 2, 128], BF, tag="twout")
            nc.sync.dma_start(out=twout[:], in_=wout2[:])
            tlam = pers.tile([128, 4, 4], F, tag="tlam")
            nc.sync.dma_start(out=tlam[:], in_=lamq.rearrange("p (a d) -> p a d", a=4))
            tgam = pers.tile([128, 4, 2, 32], F, tag="tgam")
            nc.sync.dma_start(out=tgam[:], in_=gamq.rearrange("p (a m c) -> p a m c", a=4, m=2))
            tbet = pers.tile([128, 4, 2, 32], F, tag="tbet")
            nc.sync.dma_start(out=tbet[:], in_=betq.rearrange("p (a m c) -> p a m c", a=4, m=2))

            econst = pers.tile([128, 1], F, tag="econst")
            nc.vector.memset(econst[:], math.e)
            nhalf = pers.tile([128, 1], F, tag="nhalf")
            nc.vector.memset(nhalf[:], -0.5)

            zrow = pers.tile([1, 512], BF, tag="zrow")
            nc.vector.memset(zrow[:], 0.0)

            qTp = pers.tile([128, N], R, tag="qTp")
            kTp = pers.tile([128, N], R, tag="kTp")
            vpp = pers.tile([128, 16, 66], BF, tag="vpp")
            nc.vector.memset(vpp[:, :, 32:66:33], 1.0)  # ones cols
            tg = pers.tile([128, 16, 64], F, tag="tg")

            # ---------------- projections ----------------
            def proj_qk(dst, w, it):
                ps = dp.tile([128, 1024], F, tag="dots")
                for f in range(2):
                    nc.tensor.matmul(ps[:, 0:512], w[:, f, :],
                                     xt[:, f, it * 512:(it + 1) * 512],
                                     start=(f == 0), stop=(f == 1),
                                     tile_position=(0, 0))
                nc.vector.tensor_copy(dst[:, it * 512:(it + 1) * 512], ps[:, 0:512])

            def proj_vg(t):
                # 4 position blocks of 128 in one psum tile, 256 cols each
                ps = dp.tile([128, 1024], F, tag="dots")
                for i in range(4):
                    jc = 4 * t + i
                    for f in range(2):
                        nc.tensor.matmul(ps[:, 256 * i:256 * i + 256],
                                         xt[:, f, jc * 128:(jc + 1) * 128],
                                         twvg[:, f, :],
                                         start=(f == 0), stop=False,
                                         tile_position=(0, 0))
                    nc.tensor.matmul(ps[:, 256 * i:256 * i + 256],
                                     tones[:], tbg[:],
                                     start=False, stop=True,
                                     tile_position=(0, 0))
                psv = ps[:].rearrange("p (i mc) -> p i mc", i=4)
                # v -> vpp (bf16) at [33m : 33m+32]
                nc.vector.tensor_copy(
                    vpp[:, 4 * t:4 * t + 4, :].rearrange("p i (m c) -> p i m c", m=2)[:, :, :, 0:32],
                    psv[:, :, 0:64].rearrange("p i (m c) -> p i m c", m=2))
                # tanh(0.5*(g+bg)) -> tg
                nc.scalar.activation(tg[:, 4 * t:4 * t + 4, :], psv[:, :, 64:128],
                                     AF.Tanh, scale=0.5)

            # ---------------- attention ----------------
            saved = {}

            def attention(b, interleave=None, pulls=None):
                q0 = b * NQ
                Z = zp.tile([128, 1024], F, tag="z")
                # zero both banks once: start_tensor_calc marks the whole
                # 2KB zero-region pending, so interleaved accumulation groups
                # must all run with start=False on a pre-zeroed bank.
                for bank in range(2):
                    nc.tensor.matmul(Z[:, 512 * bank:512 * bank + 512],
                                     zrow[:, 0:128], zrow[:],
                                     start=True, stop=True,
                                     tile_position=(0, 0),
                                     skip_group_check=True)
                ti = 0
                for jc in range(16):
                    if pulls is not None and jc in pulls:
                        for fn in pulls[jc]:
                            fn()
                    for p in range(2):
                        dt = dp.tile([128, 1024], F, tag="dots")
                        for dd in range(2):
                            d = 2 * p + dd
                            nc.tensor.matmul(
                                dt[:, dd * 512:(dd + 1) * 512],
                                kTp[32 * d:32 * d + 16, jc * 128:(jc + 1) * 128],
                                qTp[32 * d:32 * d + 16, q0:q0 + 512],
                                start=True, stop=True,
                                tile_position=(32 * d, 0))
                        eb = ebp.tile([128, 1024], BF, tag="eb")
                        if (b * 32 + ti) in pool_set:
                            sc = scp.tile([128, 1024], F, tag="sc")
                            nc.vector.tensor_copy(sc[:], dt[:])
                            nc.gpsimd.tensor_tensor(
                                eb[:], econst[:].broadcast_to([128, 1024]),
                                sc[:], AL.pow)
                        else:
                            nc.scalar.activation(eb[:], dt[:], AF.Exp)
                        for dd in range(2):
                            d = 2 * p + dd
                            for qs in range(4):
                                nc.tensor.matmul(
                                    Z[:, 256 * qs + 33 * d:256 * qs + 33 * d + 33],
                                    eb[:, dd * 512 + qs * 128:dd * 512 + (qs + 1) * 128],
                                    vpp[:, jc, 33 * p:33 * p + 33],
                                    start=False, stop=(jc == 15),
                                    tile_position=(0, 0),
                                    skip_group_check=True)
                        ti += 1
                        if interleave is not None and (ti % 3 == 0):
                            next(interleave, None)
                while interleave is not None:
                    try:
                        next(interleave)
                    except StopIteration:
                        break
                zs = epi.tile([128, 4, 132], F, tag="zs")
                nc.vector.tensor_copy(
                    zs[:], Z[:].rearrange("p (a w) -> p a w", a=4)[:, :, 0:132])
                if DBG:
                    nc.sync.dma_start(out=dzs[b], in_=zs[:])
                saved[b] = zs

            def epilogue(b):
                if EPI_LEVEL < 1:
                    return
                zs = saved[b]
                zv = zs[:].rearrange("p a (d w) -> p a d w", d=4)
                rinv = epi.tile([128, 4, 4], F, tag="rinv")
                nc.vector.reciprocal(rinv[:], zs[:, :, 32:132:33])
                nc.vector.tensor_tensor(rinv[:], rinv[:], tlam[:], AL.mult)
                yield
                if EPI_LEVEL < 2:
                    return
                tt = epi.tile([128, 4, 4, 32], F, tag="tt")
                (nc.gpsimd if EPI_POOL else nc.vector).tensor_tensor(tt[:], zv[:, :, :, 0:32],
                                        rinv[:].broadcast_to([128, 4, 4, 32]),
                                        AL.mult)
                yield
                # u_m = t_{2m} - t_{2m+1}: even/odd d via (m, e*c) split
                t4 = tt[:].rearrange("p a (m e) c -> p a m (e c)", m=2)
                uu = epi.tile([128, 4, 2, 32], F, tag="uu")
                nc.vector.tensor_tensor(uu[:], t4[:, :, :, 0:32], t4[:, :, :, 32:64],
                                        AL.subtract)
                if DBG:
                    nc.sync.dma_start(out=duu[b], in_=uu[:])
                yield
                if EPI_LEVEL < 3:
                    return
                st = epi.tile([128, 4, 2, 8], F, tag="st")  # stats slots
                nc.vector.tensor_reduce(st[:, :, :, 0:1], uu[:], AX.X, AL.add)
                sq = epi.tile([128, 4, 2, 32], F, tag="sq")
                (nc.gpsimd if EPI_POOL else nc.vector).tensor_tensor(sq[:], uu[:], uu[:], AL.mult)
                yield
                nc.vector.tensor_reduce(st[:, :, :, 1:2], sq[:], AX.X, AL.add)
                # mu = sum/32 ; e2 = sumsq/32 ; var = e2 - mu^2 (+eps)
                nc.vector.tensor_scalar(st[:, :, :, 2:3], st[:, :, :, 0:1], 1.0 / 32, None, AL.mult)
                nc.vector.tensor_scalar(st[:, :, :, 3:4], st[:, :, :, 1:2], 1.0 / 32, None, AL.mult)
                yield
                nc.vector.tensor_tensor(st[:, :, :, 4:5], st[:, :, :, 2:3], st[:, :, :, 2:3], AL.mult)
                nc.vector.tensor_tensor(st[:, :, :, 5:6], st[:, :, :, 3:4], st[:, :, :, 4:5], AL.subtract)
                nc.vector.tensor_scalar(st[:, :, :, 6:7], st[:, :, :, 5:6], LN_EPS, None, AL.add)
                # rs = (var+eps)^-0.5 on gpsimd
                nc.gpsimd.tensor_tensor(st[:, :, :, 7:8], st[:, :, :, 6:7],
                                        nhalf[:].broadcast_to([128, 4, 2, 1]),
                                        AL.pow)
                yield
                if EPI_LEVEL < 4:
                    return
                xb = epi.tile([128, 4, 2, 32], F, tag="xb")
                nc.vector.tensor_tensor(xb[:], uu[:],
                                        st[:, :, :, 2:3].broadcast_to([128, 4, 2, 32]),
                                        AL.subtract)
                nc.vector.tensor_tensor(xb[:], xb[:],
                                        st[:, :, :, 7:8].broadcast_to([128, 4, 2, 32]),
                                        AL.mult)
                yield
                nc.vector.tensor_tensor(xb[:], xb[:], tgam[:], AL.mult)
                nc.vector.tensor_tensor(xb[:], xb[:], tbet[:], AL.add)
                yield
                tgv = tg[:, 4 * b:4 * b + 4, :].rearrange("p a (m c) -> p a m c", m=2)
                gm = epi.tile([128, 4, 2, 32], F, tag="gm")
                (nc.gpsimd if EPI_POOL else nc.vector).tensor_tensor(gm[:], xb[:], tgv, AL.mult)
                yield
                if EPI_LEVEL < 5:
                    return
                gatedp = epi.tile([128, 2, 128], BF, tag="gatedp")
                xv = xb[:].rearrange("p (P k) m c -> p P (k m c)", P=2)
                mv = gm[:].rearrange("p (P k) m c -> p P (k m c)", P=2)
                for k in range(2):
                    (nc.gpsimd if EPI_POOL else nc.vector).tensor_tensor(gatedp[:, :, 64 * k:64 * k + 64],
                                            xv[:, :, 64 * k:64 * k + 64],
                                            mv[:, :, 64 * k:64 * k + 64], AL.add)
                yield
                if EPI_LEVEL < 6:
                    return
                gT = []
                for P in range(2):
                    g_t = epi.tile([128, 128], BF, tag=f"gT{P}", name=f"gT{P}")
                    nc.sync.dma_start(out=g_t[:], in_=gatedp[:, P, :], transpose=True)
                    gT.append(g_t)
                yield
                if EPI_LEVEL < 7:
                    return
                yp = dp.tile([128, 1024], F, tag="dots")
                for qs in range(4):
                    P, k = qs >> 1, qs & 1
                    for oh in range(2):
                        nc.tensor.matmul(
                            yp[:, 512 * k + 256 * oh + 128 * P:512 * k + 256 * oh + 128 * P + 128],
                            twout[64 * k:64 * k + 64, oh, :],
                            gT[P][64 * k:64 * k + 64, :],
                            start=True, stop=True,
                            tile_position=(64 * k, 0),
                            skip_group_check=True)
                yield
                ys = epi.tile([128, 1024], F, tag="ys")
                nc.vector.tensor_copy(ys[:], yp[:])
                if EPI_LEVEL < 8:
                    return
                ysv = ys[:].rearrange("p (k oh P q) -> p k oh P q", k=2, oh=2, P=2)
                for oh in range(2):
                    for P in range(2):
                        nc.sync.dma_start(
                            out=yT[128 * oh:128 * oh + 128,
                                   512 * b + 256 * P:512 * b + 256 * P + 256],
                            in_=ysv[:, :, oh, P, :])
                yield

            # ---------------- schedule ----------------
            proj_qk(kTp, twkp, 0)
            proj_qk(qTp, twqp, 0)
            proj_vg(0)

            pulls0 = {
                3: [lambda: proj_qk(kTp, twkp, 1), lambda: proj_vg(1)],
                7: [lambda: proj_qk(kTp, twkp, 2), lambda: proj_vg(2),
                    lambda: proj_qk(qTp, twqp, 1)],
                11: [lambda: proj_qk(kTp, twkp, 3), lambda: proj_vg(3),
                     lambda: proj_qk(qTp, twqp, 2)],
                14: [lambda: proj_qk(qTp, twqp, 3)],
            }
            attention(0, pulls=pulls0)
            for b in range(1, NB):
                attention(b, interleave=epilogue(b - 1))
            for _ in epilogue(NB - 1):
                pass
            if DBG:
                nc.sync.dma_start(out=dtg[:], in_=tg[:])

    nc.finalize()
    return nc


def _prep_core_inputs(inputs, bi, hg, lam):
    scale = DH ** -0.5
    li = np.float32(1.0 - LAMBDA_INIT)
    x = np.asarray(inputs["x"], np.float32)
    Wq = np.asarray(inputs["Wq"], np.float32)
    Wkv = np.asarray(inputs["Wkv"], np.float32)
    Wout = np.asarray(inputs["Wout"], np.float32)
    Wg = np.asarray(inputs["Wg"], np.float32)
    bg = np.asarray(inputs["bg"], np.float32)
    g_ = np.asarray(inputs["ln_gamma"], np.float32)
    b_ = np.asarray(inputs["ln_beta"], np.float32)

    c0 = 64 * hg
    wq_c = Wq[:, c0:c0 + 64] * scale
    wk_c = Wkv[:, c0:c0 + 64]
    wv_c = Wkv[:, 256 + c0:256 + c0 + 64]
    wg_c = Wg[:, c0:c0 + 64]
    wout_c = Wout[c0:c0 + 64, :]

    wqp = np.zeros((256, 128), np.float32)
    wkp = np.zeros((256, 128), np.float32)
    for d in range(4):
        wqp[:, 32 * d:32 * d + 16] = wq_c[:, 16 * d:16 * d + 16]
        wkp[:, 32 * d:32 * d + 16] = wk_c[:, 16 * d:16 * d + 16]

    wvg = np.zeros((256, 256), np.float32)
    wvg[:, 0:64] = wv_c
    wvg[:, 64:128] = wg_c

    bgrow = np.zeros((1, 256), np.float32)
    bgrow[0, 64:128] = bg[c0:c0 + 64]

    wout2 = np.zeros((128, 2, 128), np.float32)
    for k in range(2):
        for oh in range(2):
            wout2[64 * k:64 * k + 64, oh, :] = wout_c[:, 128 * oh:128 * oh + 128]

    lamq = np.tile(np.array([1.0, lam, 1.0, lam], np.float32), 4)[None, :].repeat(128, 0)
    gam1 = np.tile(g_[0:32] * li * 0.5, 8)[None, :].repeat(128, 0)
    bet1 = np.tile(b_[0:32] * li * 0.5, 8)[None, :].repeat(128, 0)

    return {
        "xT": np.ascontiguousarray(x[bi].T),
        "wqp": wqp, "wkp": wkp, "wvg": wvg,
        "bgrow": bgrow,
        "onesrow": np.ones((1, 128), np.float32),
        "wout2": wout2.astype(ml_dtypes.bfloat16),
        "lamq": np.ascontiguousarray(lamq),
        "gamq": np.ascontiguousarray(gam1),
        "betq": np.ascontiguousarray(bet1),
    }


def kernel(**inputs) -> np.ndarray:
    lq1 = np.asarray(inputs["lq1"], np.float64)
    lk1 = np.asarray(inputs["lk1"], np.float64)
    lq2 = np.asarray(inputs["lq2"], np.float64)
    lk2 = np.asarray(inputs["lk2"], np.float64)
    lam = float(np.exp(np.sum(lq1 * lk1)) - np.exp(np.sum(lq2 * lk2)) + LAMBDA_INIT)
    bout = np.asarray(inputs["bout"], np.float32)

    if "nc" not in _cached:
        _cached["nc"] = build_kernel()
    nc = _cached["nc"]

    in_maps = []
    for c in range(NC):
        bi, hg = c // 4, c % 4
        in_maps.append(_prep_core_inputs(inputs, bi, hg, lam))

    trace = bool(int(os.environ.get("BASS_KERNEL_TRACE", "0")))
    res = run_bass_kernel_spmd(nc, in_maps, list(range(NC)), trace=trace)
    _cached["exec_time_ns"] = res.exec_time_ns
    _cached["trace"] = res.instructions_and_trace
    _cached["res"] = res
    out = np.zeros((B, N, DIM), np.float32)
    for c in range(NC):
        bi = c // 4
        out[bi] += res.results[c]["yT"].T
    out += bout
    return out


# revision 30
# speedup vs baseline: 1.7735x; 1.0237x over previous
"""DiffAttn2d TRN2 Bass kernel (v2: query-major epilogue + ACT/Pool exp split).

Sharding: 8 cores = 2 (batch) x 4 (head-groups of 2 heads / 4 doubled-heads).
Per core (4 dheads d=0..3, heads m=d//2), n=2048 queries in 4 blocks of 512:
  - scores^T [keys, queries] via row-packed K=16 fp32r matmuls (baseline style)
  - exp split between ACT (direct psum->sbuf) and GPSIMD pow(e,x) (after a DVE
    psum->sbuf staging copy); both write bf16
  - attn@v TRANSPOSED: stationary = exp-scores slice [128k, 128q], moving =
    [v_m | 1] (33 cols) -> Z[q, (d,c)] in psum, softmax denominators free.
    Cost ~33 cols/slice instead of 512.
  - epilogue in query-major on DVE/Pool: reciprocal denominators, lambda fold,
    pair subtraction, LayerNorm stats via free-dim reduce, rsqrt via pow(x,-.5)
    on Pool, gating via tanh (sigma = .5 tanh(x/2)+.5, folded into gamma/beta)
  - gated -> bf16, DMA-transposed ([128q,128c'] -> [c',q]) and projected with a
    partition-duplicated Wout (K=64, tile rows 64k -> psum bank k)
  - yT partials summed on host (+ bout)
"""
import sys
sys.path.insert(0, "/opt/trn_rl_repo")

import math
import os
import numpy as np
import ml_dtypes

import concourse.bass as bass
import concourse.bacc as bacc_mod
import concourse.mybir as mybir
from concourse.tile import TileContext
from concourse.bass_utils import run_bass_kernel_spmd

F = mybir.dt.float32
R = mybir.dt.float32r
BF = mybir.dt.bfloat16
AF = mybir.ActivationFunctionType
AL = mybir.AluOpType
AX = mybir.AxisListType

H, DH = 8, 16
DEPTH = 1
LAMBDA_INIT = 0.8 - 0.6 * math.exp(-0.3 * DEPTH)
LN_EPS = 1e-5
B, N, DIM = 2, 2048, 256
NC = 8

NB = 4          # query blocks per core
NQ = 512        # queries per block
NTILE = NB * 16 * 2  # exp tiles total (b, jc, pair)
import os as _os
NPOOL = int(_os.environ.get("NPOOL", "54"))
EPI_LEVEL = int(_os.environ.get("EPI_LEVEL", "9"))
EPI_POOL = int(_os.environ.get("EPI_POOL", "1"))
PULL_AT = {4, 8, 11, 14, 17, 20, 22, 24, 26, 28, 30, 31}
DBG = int(_os.environ.get("KDBG", "0"))

_cached = {}


def _pool_set():
    if NPOOL <= 0:
        return set()
    return {int(round(j * NTILE / NPOOL)) for j in range(NPOOL)}


def build_kernel():
    nc = bacc_mod.Bacc()
    xT = nc.declare_dram_parameter("xT", [DIM, N], R, isOutput=False)
    wqp = nc.declare_dram_parameter("wqp", [DIM, 128], R, isOutput=False)
    wkp = nc.declare_dram_parameter("wkp", [DIM, 128], R, isOutput=False)
    wvg = nc.declare_dram_parameter("wvg", [DIM, 256], R, isOutput=False)
    bgrow = nc.declare_dram_parameter("bgrow", [1, 256], R, isOutput=False)
    onesrow = nc.declare_dram_parameter("onesrow", [1, 128], R, isOutput=False)
    wout2 = nc.declare_dram_parameter("wout2", [128, 2, 128], BF, isOutput=False)
    lamq = nc.declare_dram_parameter("lamq", [128, 16], F, isOutput=False)
    ident = nc.declare_dram_parameter("ident", [128, 128], BF, isOutput=False)
    gamq = nc.declare_dram_parameter("gamq", [128, 256], F, isOutput=False)
    betq = nc.declare_dram_parameter("betq", [128, 256], F, isOutput=False)
    yT = nc.declare_dram_parameter("yT", [DIM, N], F, isOutput=True)
    dzs = nc.declare_dram_parameter("dzs", [NB, 128, 4, 132], F, isOutput=True)
    duu = nc.declare_dram_parameter("duu", [NB, 128, 4, 2, 32], F, isOutput=True)
    dgp = nc.declare_dram_parameter("dgp", [NB, 128, 2, 128], F, isOutput=True)
    dtg = nc.declare_dram_parameter("dtg", [128, 16, 64], F, isOutput=True)

    pool_set = _pool_set()

    with TileContext(nc) as tc:
        with tc.tile_pool(name="pers", bufs=1) as pers, \
             tc.tile_pool(name="ebp", bufs=8) as ebp, \
             tc.tile_pool(name="scp", bufs=6) as scp, \
             tc.tile_pool(name="epi", bufs=2) as epi, \
             tc.tile_pool(name="dp", bufs=3, space="PSUM") as dp, \
             tc.tile_pool(name="zp", bufs=1, space="PSUM") as zp:

            # prewarm the exp/tanh ACT table during the DMA window
            warm = pers.tile([1, 8], F, tag="warm")
            nc.vector.memset(warm[:], 0.0)
            nc.scalar.activation(warm[:], warm[:], AF.Exp)

            # ---------------- persistent loads ----------------
            xt = pers.tile([128, 2, N], R, tag="xt")
            xTr = xT.rearrange("(f p) n -> p f n", p=128)
            for f in range(2):
                nc.sync.dma_start(out=xt[:, f, 0:512], in_=xTr[:, f, 0:512])
            twkp = pers.tile([128, 2, 128], R, tag="twkp")
            nc.sync.dma_start(out=twkp[:], in_=wkp.rearrange("(f p) m -> p f m", p=128))
            twqp = pers.tile([128, 2, 128], R, tag="twqp")
            nc.sync.dma_start(out=twqp[:], in_=wqp.rearrange("(f p) m -> p f m", p=128))
            twvg = pers.tile([128, 2, 256], R, tag="twvg")
            nc.sync.dma_start(out=twvg[:], in_=wvg.rearrange("(f p) m -> p f m", p=128))
            tbg = pers.tile([1, 256], R, tag="tbg")
            nc.sync.dma_start(out=tbg[:], in_=bgrow[:])
            tones = pers.tile([1, 128], R, tag="tones")
            nc.sync.dma_start(out=tones[:], in_=onesrow[:])
            for n0 in range(512, 2048, 512):
                for f in range(2):
                    nc.sync.dma_start(out=xt[:, f, n0:n0 + 512],
                                      in_=xTr[:, f, n0:n0 + 512])
            twout = pers.tile([128, 2, 128], BF, tag="twout")
            nc.sync.dma_start(out=twout[:], in_=wout2[:])
            tident = pers.tile([128, 128], BF, tag="tident")
            nc.sync.dma_start(out=tident[:], in_=ident[:])
            tlam = pers.tile([128, 4, 4], F, tag="tlam")
            nc.sync.dma_start(out=tlam[:], in_=lamq.rearrange("p (a d) -> p a d", a=4))
            tgam = pers.tile([128, 4, 2, 32], F, tag="tgam")
            nc.sync.dma_start(out=tgam[:], in_=gamq.rearrange("p (a m c) -> p a m c", a=4, m=2))
            tbet = pers.tile([128, 4, 2, 32], F, tag="tbet")
            nc.sync.dma_start(out=tbet[:], in_=betq.rearrange("p (a m c) -> p a m c", a=4, m=2))

            econst = pers.tile([128, 1], F, tag="econst")
            nc.vector.memset(econst[:], math.e)
            nhalf = pers.tile([128, 1], F, tag="nhalf")
            nc.vector.memset(nhalf[:], -0.5)

            zrow = pers.tile([1, 512], BF, tag="zrow")
            nc.vector.memset(zrow[:], 0.0)

            qTp = pers.tile([128, N], R, tag="qTp")
            kTp = pers.tile([128, N], R, tag="kTp")
            vpp = pers.tile([128, 16, 66], BF, tag="vpp")
            nc.vector.memset(vpp[:, :, 32:66:33], 1.0)  # ones cols
            tg = pers.tile([128, 16, 64], F, tag="tg")

            # ---------------- projections ----------------
            def proj_qk(dst, w, it):
                ps = dp.tile([128, 1024], F, tag="dots")
                for f in range(2):
                    nc.tensor.matmul(ps[:, 0:512], w[:, f, :],
                                     xt[:, f, it * 512:(it + 1) * 512],
                                     start=(f == 0), stop=(f == 1),
                                     tile_position=(0, 0))
                nc.vector.tensor_copy(dst[:, it * 512:(it + 1) * 512], ps[:, 0:512])

            def proj_vg(t):
                # 4 position blocks of 128 in one psum tile, 256 cols each
                ps = dp.tile([128, 1024], F, tag="dots")
                for i in range(4):
                    jc = 4 * t + i
                    for f in range(2):
                        nc.tensor.matmul(ps[:, 256 * i:256 * i + 256],
                                         xt[:, f, jc * 128:(jc + 1) * 128],
                                         twvg[:, f, :],
                                         start=(f == 0), stop=False,
                                         tile_position=(0, 0))
                    nc.tensor.matmul(ps[:, 256 * i:256 * i + 256],
                                     tones[:], tbg[:],
                                     start=False, stop=True,
                                     tile_position=(0, 0))
                psv = ps[:].rearrange("p (i mc) -> p i mc", i=4)
                # v -> vpp (bf16) at [33m : 33m+32]
                nc.vector.tensor_copy(
                    vpp[:, 4 * t:4 * t + 4, :].rearrange("p i (m c) -> p i m c", m=2)[:, :, :, 0:32],
                    psv[:, :, 0:64].rearrange("p i (m c) -> p i m c", m=2))
                # tanh(0.5*(g+bg)) -> tg
                nc.scalar.activation(tg[:, 4 * t:4 * t + 4, :], psv[:, :, 64:128],
                                     AF.Tanh, scale=0.5)

            # ---------------- attention ----------------
            saved = {}

            def attention(b, interleave=None, pulls=None):
                q0 = b * NQ
                Z = zp.tile([128, 1024], F, tag="z")
                # zero both banks once: start_tensor_calc marks the whole
                # 2KB zero-region pending, so interleaved accumulation groups
                # must all run with start=False on a pre-zeroed bank.
                for bank in range(2):
                    nc.tensor.matmul(Z[:, 512 * bank:512 * bank + 512],
                                     zrow[:, 0:128], zrow[:],
                                     start=True, stop=True,
                                     tile_position=(0, 0),
                                     skip_group_check=True)
                def attnv(eb, jc, p):
                    for dd in range(2):
                        d = 2 * p + dd
                        for qs in range(4):
                            nc.tensor.matmul(
                                Z[:, 256 * qs + 33 * d:256 * qs + 33 * d + 33],
                                eb[:, dd * 512 + qs * 128:dd * 512 + (qs + 1) * 128],
                                vpp[:, jc, 33 * p:33 * p + 33],
                                start=False, stop=(jc == 15),
                                tile_position=(0, 0),
                                skip_group_check=True)

                ti = 0
                pend = None
                for jc in range(16):
                    if pulls is not None and jc in pulls:
                        for fn in pulls[jc]:
                            fn()
                    for p in range(2):
                        dt = dp.tile([128, 1024], F, tag="dots")
                        for dd in range(2):
                            d = 2 * p + dd
                            nc.tensor.matmul(
                                dt[:, dd * 512:(dd + 1) * 512],
                                kTp[32 * d:32 * d + 16, jc * 128:(jc + 1) * 128],
                                qTp[32 * d:32 * d + 16, q0:q0 + 512],
                                start=True, stop=True,
                                tile_position=(32 * d, 0))
                        eb = ebp.tile([128, 1024], BF, tag="eb")
                        if (b * 32 + ti) in pool_set:
                            sc = scp.tile([128, 1024], F, tag="sc")
                            nc.vector.tensor_copy(sc[:], dt[:])
                            nc.gpsimd.tensor_tensor(
                                eb[:], econst[:].broadcast_to([128, 1024]),
                                sc[:], AL.pow)
                        else:
                            nc.scalar.activation(eb[:], dt[:], AF.Exp)
                        # software-pipelined: emit the PREVIOUS tile's attn@v
                        # after this tile's scores so the in-order PE stream
                        # never stalls on a still-running exp.
                        if pend is not None:
                            attnv(*pend)
                        pend = (eb, jc, p)
                        ti += 1
                        if interleave is not None and ti in PULL_AT:
                            next(interleave, None)
                attnv(*pend)
                while interleave is not None:
                    try:
                        next(interleave)
                    except StopIteration:
                        break
                zs = epi.tile([128, 4, 132], F, tag="zs")
                nc.vector.tensor_copy(
                    zs[:], Z[:].rearrange("p (a w) -> p a w", a=4)[:, :, 0:132])
                if DBG:
                    nc.sync.dma_start(out=dzs[b], in_=zs[:])
                saved[b] = zs

            def epi_chain(b, lo, hi, eng, gatedp):
                """LN/gate math for qs in [lo, hi). eng: 'm' mixed, 'v' DVE, 'g' Pool."""
                w = hi - lo
                V, G = nc.vector, nc.gpsimd
                if eng == "v":
                    e_tt = e_sq = e_xb = e_gb = e_gm = e_gp = V
                elif eng == "g":
                    e_tt = e_sq = e_xb = e_gb = e_gm = e_gp = G
                else:
                    e_tt = e_sq = e_xb = e_gb = e_gm = e_gp = (G if EPI_POOL else V)
                zs = saved[b]
                zv = zs[:, lo:hi].rearrange("p a (d e) -> p a d e", d=4)
                tag = f"_{lo}_{w}"
                rinv = epi.tile([128, w, 4], F, tag="rinv" + tag)
                nc.vector.reciprocal(rinv[:], zs[:, lo:hi, 32:132:33])
                nc.vector.tensor_tensor(rinv[:], rinv[:], tlam[:, lo:hi], AL.mult)
                yield
                tt = epi.tile([128, w, 4, 32], F, tag="tt" + tag)
                e_tt.tensor_tensor(tt[:], zv[:, :, :, 0:32],
                                   rinv[:].broadcast_to([128, w, 4, 32]), AL.mult)
                yield
                # u_m = t_{2m} - t_{2m+1}: even/odd d via (m, e*c) split
                t4 = tt[:].rearrange("p a (m e) c -> p a m (e c)", m=2)
                uu = epi.tile([128, w, 2, 32], F, tag="uu" + tag)
                nc.vector.tensor_tensor(uu[:], t4[:, :, :, 0:32], t4[:, :, :, 32:64],
                                        AL.subtract)
                yield
                st = epi.tile([128, w, 2, 8], F, tag="st" + tag)  # stats slots
                nc.vector.tensor_reduce(st[:, :, :, 0:1], uu[:], AX.X, AL.add)
                sq = epi.tile([128, w, 2, 32], F, tag="sq" + tag)
                e_sq.tensor_tensor(sq[:], uu[:], uu[:], AL.mult)
                yield
                nc.vector.tensor_reduce(st[:, :, :, 1:2], sq[:], AX.X, AL.add)
                # mu = sum/32 ; e2 = sumsq/32 ; var = e2 - mu^2 (+eps)
                nc.vector.tensor_scalar(st[:, :, :, 2:3], st[:, :, :, 0:1], 1.0 / 32, None, AL.mult)
                nc.vector.tensor_scalar(st[:, :, :, 3:4], st[:, :, :, 1:2], 1.0 / 32, None, AL.mult)
                yield
                nc.vector.tensor_tensor(st[:, :, :, 4:5], st[:, :, :, 2:3], st[:, :, :, 2:3], AL.mult)
                nc.vector.tensor_tensor(st[:, :, :, 5:6], st[:, :, :, 3:4], st[:, :, :, 4:5], AL.subtract)
                nc.vector.tensor_scalar(st[:, :, :, 6:7], st[:, :, :, 5:6], LN_EPS, None, AL.add)
                # rs = (var+eps)^-0.5 on gpsimd
                nc.gpsimd.tensor_tensor(st[:, :, :, 7:8], st[:, :, :, 6:7],
                                        nhalf[:].broadcast_to([128, w, 2, 1]),
                                        AL.pow)
                yield
                xb = epi.tile([128, w, 2, 32], F, tag="xb" + tag)
                e_xb.tensor_tensor(xb[:], uu[:],
                                   st[:, :, :, 2:3].broadcast_to([128, w, 2, 32]), AL.subtract)
                e_xb.tensor_tensor(xb[:], xb[:],
                                   st[:, :, :, 7:8].broadcast_to([128, w, 2, 32]), AL.mult)
                yield
                e_gb.tensor_tensor(xb[:], xb[:], tgam[:, lo:hi], AL.mult)
                e_gb.tensor_tensor(xb[:], xb[:], tbet[:, lo:hi], AL.add)
                yield
                tgv = tg[:, 4 * b + lo:4 * b + hi, :].rearrange("p a (m c) -> p a m c", m=2)
                gm = epi.tile([128, w, 2, 32], F, tag="gm" + tag)
                e_gm.tensor_tensor(gm[:], xb[:], tgv, AL.mult)
                yield
                xv = xb[:].rearrange("p (P k) m c -> p P (k m c)", P=w // 2)
                mv = gm[:].rearrange("p (P k) m c -> p P (k m c)", P=w // 2)
                gp = gatedp[:, lo // 2:hi // 2]
                for k in range(2):
                    e_gp.tensor_tensor(gp[:, :, 64 * k:64 * k + 64],
                                       xv[:, :, 64 * k:64 * k + 64],
                                       mv[:, :, 64 * k:64 * k + 64], AL.add)
                yield

            def epilogue(b):
                gatedp = epi.tile([128, 2, 128], BF, tag="gatedp")
                if b < NB - 1:
                    for _ in epi_chain(b, 0, 4, "m", gatedp):
                        yield
                else:
                    ca = epi_chain(b, 0, 2, "v", gatedp)
                    cb = epi_chain(b, 2, 4, "g", gatedp)
                    while True:
                        ra = next(ca, "END")
                        rb = next(cb, "END")
                        if ra == "END" and rb == "END":
                            break
                        yield
                gT = []
                if b < NB - 1:
                    for P in range(2):
                        g_t = epi.tile([128, 128], BF, tag=f"gT{P}", name=f"gT{P}")
                        nc.sync.dma_start(out=g_t[:], in_=gatedp[:, P, :], transpose=True)
                        gT.append(g_t)
                else:
                    # last block: PE transpose (short latency) via a dots slot
                    tp = dp.tile([128, 1024], F, tag="dots")
                    for P in range(2):
                        nc.tensor.matmul(tp[:, 64 * P:64 * P + 64].bitcast(BF),
                                         gatedp[:, P, :], tident[:],
                                         is_transpose=True,
                                         start=True, stop=True,
                                         tile_position=(0, 0),
                                         skip_group_check=True)
                    for P in range(2):
                        g_t = epi.tile([128, 128], BF, tag=f"gT{P}", name=f"gT{P}")
                        nc.vector.tensor_copy(g_t[:], tp[:, 64 * P:64 * P + 64].bitcast(BF))
                        gT.append(g_t)
                yield
                yp = dp.tile([128, 1024], F, tag="dots")
                for qs in range(4):
                    P, k = qs >> 1, qs & 1
                    for oh in range(2):
                        nc.tensor.matmul(
                            yp[:, 512 * k + 256 * oh + 128 * P:512 * k + 256 * oh + 128 * P + 128],
                            twout[64 * k:64 * k + 64, oh, :],
                            gT[P][64 * k:64 * k + 64, :],
                            start=True, stop=True,
                            tile_position=(64 * k, 0),
                            skip_group_check=True)
                yield
                ys = epi.tile([128, 1024], F, tag="ys")
                ysv = ys[:].rearrange("p (k oh P q) -> p k oh P q", k=2, oh=2, P=2)
                for k in range(2):
                    nc.vector.tensor_copy(ys[:, 512 * k:512 * k + 512],
                                          yp[:, 512 * k:512 * k + 512])
                for oh in range(2):
                    for P in range(2):
                        nc.sync.dma_start(
                            out=yT[128 * oh:128 * oh + 128,
                                   512 * b + 256 * P:512 * b + 256 * P + 256],
                            in_=ysv[:, :, oh, P, :])
                yield

            # ---------------- schedule ----------------
            proj_qk(kTp, twkp, 0)
            proj_qk(qTp, twqp, 0)
            proj_vg(0)

            pulls0 = {
                2: [lambda: proj_qk(kTp, twkp, 1)],
                3: [lambda: proj_vg(1)],
                6: [lambda: proj_qk(kTp, twkp, 2)],
                7: [lambda: proj_vg(2)],
                9: [lambda: proj_qk(qTp, twqp, 1)],
                10: [lambda: proj_qk(kTp, twkp, 3)],
                11: [lambda: proj_vg(3)],
                13: [lambda: proj_qk(qTp, twqp, 2)],
                14: [lambda: proj_qk(qTp, twqp, 3)],
            }
            attention(0, pulls=pulls0)
            for b in range(1, NB):
                attention(b, interleave=epilogue(b - 1))
            for _ in epilogue(NB - 1):
                pass
            if DBG:
                nc.sync.dma_start(out=dtg[:], in_=tg[:])

    nc.finalize()
    return nc


def _prep_core_inputs(inputs, bi, hg, lam):
    scale = DH ** -0.5
    li = np.float32(1.0 - LAMBDA_INIT)
    x = np.asarray(inputs["x"], np.float32)
    Wq = np.asarray(inputs["Wq"], np.float32)
    Wkv = np.asarray(inputs["Wkv"], np.float32)
    Wout = np.asarray(inputs["Wout"], np.float32)
    Wg = np.asarray(inputs["Wg"], np.float32)
    bg = np.asarray(inputs["bg"], np.float32)
    g_ = np.asarray(inputs["ln_gamma"], np.float32)
    b_ = np.asarray(inputs["ln_beta"], np.float32)

    c0 = 64 * hg
    wq_c = Wq[:, c0:c0 + 64] * scale
    wk_c = Wkv[:, c0:c0 + 64]
    wv_c = Wkv[:, 256 + c0:256 + c0 + 64]
    wg_c = Wg[:, c0:c0 + 64]
    wout_c = Wout[c0:c0 + 64, :]

    wqp = np.zeros((256, 128), np.float32)
    wkp = np.zeros((256, 128), np.float32)
    for d in range(4):
        wqp[:, 32 * d:32 * d + 16] = wq_c[:, 16 * d:16 * d + 16]
        wkp[:, 32 * d:32 * d + 16] = wk_c[:, 16 * d:16 * d + 16]

    wvg = np.zeros((256, 256), np.float32)
    wvg[:, 0:64] = wv_c
    wvg[:, 64:128] = wg_c

    bgrow = np.zeros((1, 256), np.float32)
    bgrow[0, 64:128] = bg[c0:c0 + 64]

    wout2 = np.zeros((128, 2, 128), np.float32)
    for k in range(2):
        for oh in range(2):
            wout2[64 * k:64 * k + 64, oh, :] = wout_c[:, 128 * oh:128 * oh + 128]

    lamq = np.tile(np.array([1.0, lam, 1.0, lam], np.float32), 4)[None, :].repeat(128, 0)
    gam1 = np.tile(g_[0:32] * li * 0.5, 8)[None, :].repeat(128, 0)
    bet1 = np.tile(b_[0:32] * li * 0.5, 8)[None, :].repeat(128, 0)

    return {
        "xT": np.ascontiguousarray(x[bi].T),
        "wqp": wqp, "wkp": wkp, "wvg": wvg,
        "bgrow": bgrow,
        "onesrow": np.ones((1, 128), np.float32),
        "ident": np.eye(128, dtype=np.float32).astype(ml_dtypes.bfloat16),
        "wout2": wout2.astype(ml_dtypes.bfloat16),
        "lamq": np.ascontiguousarray(lamq),
        "gamq": np.ascontiguousarray(gam1),
        "betq": np.ascontiguousarray(bet1),
    }


def kernel(**inputs) -> np.ndarray:
    lq1 = np.asarray(inputs["lq1"], np.float64)
    lk1 = np.asarray(inputs["lk1"], np.float64)
    lq2 = np.asarray(inputs["lq2"], np.float64)
    lk2 = np.asarray(inputs["lk2"], np.float64)
    lam = float(np.exp(np.sum(lq1 * lk1)) - np.exp(np.sum(lq2 * lk2)) + LAMBDA_INIT)
    bout = np.asarray(inputs["bout"], np.float32)

    if "nc" not in _cached:
        _cached["nc"] = build_kernel()
    nc = _cached["nc"]

    in_maps = []
    for c in range(NC):
        bi, hg = c // 4, c % 4
        in_maps.append(_prep_core_inputs(inputs, bi, hg, lam))

    trace = bool(int(os.environ.get("BASS_KERNEL_TRACE", "0")))
    res = run_bass_kernel_spmd(nc, in_maps, list(range(NC)), trace=trace)
    _cached["exec_time_ns"] = res.exec_time_ns
    _cached["trace"] = res.instructions_and_trace
    _cached["res"] = res
    out = np.zeros((B, N, DIM), np.float32)
    for c in range(NC):
        bi = c // 4
        out[bi] += res.results[c]["yT"].T
    out += bout
    return out


# revision 35
# speedup vs baseline: 1.8016x; 1.0158x over previous
"""DiffAttn2d TRN2 Bass kernel (v2: query-major epilogue + ACT/Pool exp split).

Sharding: 8 cores = 2 (batch) x 4 (head-groups of 2 heads / 4 doubled-heads).
Per core (4 dheads d=0..3, heads m=d//2), n=2048 queries in 4 blocks of 512:
  - scores^T [keys, queries] via row-packed K=16 fp32r matmuls (baseline style)
  - exp split between ACT (direct psum->sbuf) and GPSIMD pow(e,x) (after a DVE
    psum->sbuf staging copy); both write bf16
  - attn@v TRANSPOSED: stationary = exp-scores slice [128k, 128q], moving =
    [v_m | 1] (33 cols) -> Z[q, (d,c)] in psum, softmax denominators free.
    Cost ~33 cols/slice instead of 512.
  - epilogue in query-major on DVE/Pool: reciprocal denominators, lambda fold,
    pair subtraction, LayerNorm stats via free-dim reduce, rsqrt via pow(x,-.5)
    on Pool, gating via tanh (sigma = .5 tanh(x/2)+.5, folded into gamma/beta)
  - gated -> bf16, DMA-transposed ([128q,128c'] -> [c',q]) and projected with a
    partition-duplicated Wout (K=64, tile rows 64k -> psum bank k)
  - yT partials summed on host (+ bout)
"""
import sys
sys.path.insert(0, "/opt/trn_rl_repo")

import math
import os
import numpy as np
import ml_dtypes

import concourse.bass as bass
import concourse.bacc as bacc_mod
import concourse.mybir as mybir
from concourse.tile import TileContext
from concourse.bass_utils import run_bass_kernel_spmd

F = mybir.dt.float32
R = mybir.dt.float32r
BF = mybir.dt.bfloat16
AF = mybir.ActivationFunctionType
AL = mybir.AluOpType
AX = mybir.AxisListType

H, DH = 8, 16
DEPTH = 1
LAMBDA_INIT = 0.8 - 0.6 * math.exp(-0.3 * DEPTH)
LN_EPS = 1e-5
B, N, DIM = 2, 2048, 256
NC = 8

NB = 4          # query blocks per core
NQ = 512        # queries per block
NTILE = NB * 16 * 2  # exp tiles total (b, jc, pair)
import os as _os
NPOOL = int(_os.environ.get("NPOOL", "52"))
EPI_LEVEL = int(_os.environ.get("EPI_LEVEL", "9"))
EPI_POOL = int(_os.environ.get("EPI_POOL", "1"))
_PA = _os.environ.get("PULLS", "a")
PULL_AT = {"a": {4, 8, 11, 14, 17, 20, 22, 24, 26, 28, 30, 31},
           "b": {2, 4, 6, 8, 10, 12, 14, 16, 18, 20, 22, 24},
           "c": {6, 9, 12, 15, 18, 21, 24, 26, 28, 30, 31, 32},
           }[_PA]
DBG = int(_os.environ.get("KDBG", "0"))

_cached = {}


def _pool_set():
    if NPOOL <= 0:
        return set()
    return {int(round(j * NTILE / NPOOL)) for j in range(NPOOL)}


def build_kernel():
    nc = bacc_mod.Bacc()
    xT = nc.declare_dram_parameter("xT", [DIM, N], R, isOutput=False)
    wqp = nc.declare_dram_parameter("wqp", [DIM, 128], R, isOutput=False)
    wkp = nc.declare_dram_parameter("wkp", [DIM, 128], R, isOutput=False)
    wvg = nc.declare_dram_parameter("wvg", [DIM, 256], R, isOutput=False)
    bgrow = nc.declare_dram_parameter("bgrow", [1, 256], R, isOutput=False)
    onesrow = nc.declare_dram_parameter("onesrow", [1, 128], R, isOutput=False)
    wout2 = nc.declare_dram_parameter("wout2", [128, 2, 128], BF, isOutput=False)
    lamq = nc.declare_dram_parameter("lamq", [128, 16], F, isOutput=False)
    ident = nc.declare_dram_parameter("ident", [128, 128], BF, isOutput=False)
    gamq = nc.declare_dram_parameter("gamq", [128, 256], F, isOutput=False)
    betq = nc.declare_dram_parameter("betq", [128, 256], F, isOutput=False)
    yT = nc.declare_dram_parameter("yT", [DIM, N], F, isOutput=True)

    pool_set = _pool_set()

    with TileContext(nc) as tc:
        with tc.tile_pool(name="pers", bufs=1) as pers, \
             tc.tile_pool(name="ebp", bufs=12) as ebp, \
             tc.tile_pool(name="scp", bufs=10) as scp, \
             tc.tile_pool(name="epi", bufs=2) as epi, \
             tc.tile_pool(name="dp", bufs=3, space="PSUM") as dp, \
             tc.tile_pool(name="zp", bufs=1, space="PSUM") as zp:

            # prewarm the exp/tanh ACT table during the DMA window
            warm = pers.tile([1, 8], F, tag="warm")
            nc.vector.memset(warm[:], 0.0)
            nc.scalar.activation(warm[:], warm[:], AF.Exp)

            # ---------------- persistent loads ----------------
            xt = pers.tile([128, 2, N], R, tag="xt")
            xTr = xT.rearrange("(f p) n -> p f n", p=128)
            for f in range(2):
                nc.sync.dma_start(out=xt[:, f, 0:512], in_=xTr[:, f, 0:512])
            twkp = pers.tile([128, 2, 128], R, tag="twkp")
            nc.sync.dma_start(out=twkp[:], in_=wkp.rearrange("(f p) m -> p f m", p=128))
            twqp = pers.tile([128, 2, 128], R, tag="twqp")
            nc.sync.dma_start(out=twqp[:], in_=wqp.rearrange("(f p) m -> p f m", p=128))
            twvg = pers.tile([128, 2, 256], R, tag="twvg")
            nc.sync.dma_start(out=twvg[:], in_=wvg.rearrange("(f p) m -> p f m", p=128))
            tbg = pers.tile([1, 256], R, tag="tbg")
            nc.sync.dma_start(out=tbg[:], in_=bgrow[:])
            tones = pers.tile([1, 128], R, tag="tones")
            nc.sync.dma_start(out=tones[:], in_=onesrow[:])
            for n0 in range(512, 2048, 512):
                for f in range(2):
                    nc.sync.dma_start(out=xt[:, f, n0:n0 + 512],
                                      in_=xTr[:, f, n0:n0 + 512])
            twout = pers.tile([128, 2, 128], BF, tag="twout")
            nc.sync.dma_start(out=twout[:], in_=wout2[:])
            tident = pers.tile([128, 128], BF, tag="tident")
            nc.sync.dma_start(out=tident[:], in_=ident[:])
            tlam = pers.tile([128, 4, 4], F, tag="tlam")
            nc.sync.dma_start(out=tlam[:], in_=lamq.rearrange("p (a d) -> p a d", a=4))
            tgam = pers.tile([128, 4, 2, 32], F, tag="tgam")
            nc.sync.dma_start(out=tgam[:], in_=gamq.rearrange("p (a m c) -> p a m c", a=4, m=2))
            tbet = pers.tile([128, 4, 2, 32], F, tag="tbet")
            nc.sync.dma_start(out=tbet[:], in_=betq.rearrange("p (a m c) -> p a m c", a=4, m=2))

            econst = pers.tile([128, 1], F, tag="econst")
            nc.vector.memset(econst[:], math.e)
            nhalf = pers.tile([128, 1], F, tag="nhalf")
            nc.vector.memset(nhalf[:], -0.5)

            zrow = pers.tile([1, 512], BF, tag="zrow")
            nc.vector.memset(zrow[:], 0.0)

            qTp = pers.tile([128, N], R, tag="qTp")
            kTp = pers.tile([128, N], R, tag="kTp")
            vpp = pers.tile([128, 16, 66], BF, tag="vpp")
            nc.vector.memset(vpp[:, :, 32:66:33], 1.0)  # ones cols
            tg = pers.tile([128, 16, 64], F, tag="tg")

            # ---------------- projections ----------------
            def proj_qk(dst, w, it):
                ps = dp.tile([128, 1024], F, tag="dots")
                for f in range(2):
                    nc.tensor.matmul(ps[:, 0:512], w[:, f, :],
                                     xt[:, f, it * 512:(it + 1) * 512],
                                     start=(f == 0), stop=(f == 1),
                                     tile_position=(0, 0))
                nc.vector.tensor_copy(dst[:, it * 512:(it + 1) * 512], ps[:, 0:512])

            def proj_vg(t):
                # 4 position blocks of 128 in one psum tile, 256 cols each
                ps = dp.tile([128, 1024], F, tag="dots")
                for i in range(4):
                    jc = 4 * t + i
                    for f in range(2):
                        nc.tensor.matmul(ps[:, 256 * i:256 * i + 256],
                                         xt[:, f, jc * 128:(jc + 1) * 128],
                                         twvg[:, f, :],
                                         start=(f == 0), stop=False,
                                         tile_position=(0, 0))
                    nc.tensor.matmul(ps[:, 256 * i:256 * i + 256],
                                     tones[:], tbg[:],
                                     start=False, stop=True,
                                     tile_position=(0, 0))
                psv = ps[:].rearrange("p (i mc) -> p i mc", i=4)
                # v -> vpp (bf16) at [33m : 33m+32]
                nc.vector.tensor_copy(
                    vpp[:, 4 * t:4 * t + 4, :].rearrange("p i (m c) -> p i m c", m=2)[:, :, :, 0:32],
                    psv[:, :, 0:64].rearrange("p i (m c) -> p i m c", m=2))
                # tanh(0.5*(g+bg)) -> tg
                nc.scalar.activation(tg[:, 4 * t:4 * t + 4, :], psv[:, :, 64:128],
                                     AF.Tanh, scale=0.5)

            # ---------------- attention ----------------
            saved = {}

            def attention(b, interleave=None, pulls=None):
                q0 = b * NQ
                Z = zp.tile([128, 1024], F, tag="z")
                # zero both banks once: start_tensor_calc marks the whole
                # 2KB zero-region pending, so interleaved accumulation groups
                # must all run with start=False on a pre-zeroed bank.
                for bank in range(2):
                    nc.tensor.matmul(Z[:, 512 * bank:512 * bank + 512],
                                     zrow[:, 0:128], zrow[:],
                                     start=True, stop=True,
                                     tile_position=(0, 0),
                                     skip_group_check=True)
                def attnv(eb, jc, p):
                    for dd in range(2):
                        d = 2 * p + dd
                        for qs in range(4):
                            nc.tensor.matmul(
                                Z[:, 256 * qs + 33 * d:256 * qs + 33 * d + 33],
                                eb[:, dd * 512 + qs * 128:dd * 512 + (qs + 1) * 128],
                                vpp[:, jc, 33 * p:33 * p + 33],
                                start=False, stop=(jc == 15),
                                tile_position=(0, 0),
                                skip_group_check=True)

                ti = 0
                pend = None
                for jc in range(16):
                    if pulls is not None and jc in pulls:
                        for fn in pulls[jc]:
                            fn()
                    for p in range(2):
                        dt = dp.tile([128, 1024], F, tag="dots")
                        for dd in range(2):
                            d = 2 * p + dd
                            nc.tensor.matmul(
                                dt[:, dd * 512:(dd + 1) * 512],
                                kTp[32 * d:32 * d + 16, jc * 128:(jc + 1) * 128],
                                qTp[32 * d:32 * d + 16, q0:q0 + 512],
                                start=True, stop=True,
                                tile_position=(32 * d, 0))
                        eb = ebp.tile([128, 1024], BF, tag="eb")
                        if (b * 32 + ti) in pool_set:
                            sc = scp.tile([128, 1024], F, tag="sc")
                            with tc.high_priority(offset=40):
                                nc.vector.tensor_copy(sc[:], dt[:])
                            nc.gpsimd.tensor_tensor(
                                eb[:], econst[:].broadcast_to([128, 1024]),
                                sc[:], AL.pow)
                        else:
                            nc.scalar.activation(eb[:], dt[:], AF.Exp)
                        # software-pipelined: emit the PREVIOUS tile's attn@v
                        # after this tile's scores so the in-order PE stream
                        # never stalls on a still-running exp.
                        if pend is not None:
                            attnv(*pend)
                        pend = (eb, jc, p)
                        ti += 1
                        if interleave is not None and ti in PULL_AT:
                            next(interleave, None)
                attnv(*pend)
                while interleave is not None:
                    try:
                        next(interleave)
                    except StopIteration:
                        break
                zs = epi.tile([128, 4, 132], F, tag="zs")
                with tc.high_priority(offset=40):
                    nc.vector.tensor_copy(
                        zs[:], Z[:].rearrange("p (a w) -> p a w", a=4)[:, :, 0:132])
                saved[b] = zs

            def epi_chain(b, lo, hi, eng, gatedp):
                """LN/gate math for qs in [lo, hi). eng: 'm' mixed, 'v' DVE, 'g' Pool."""
                w = hi - lo
                V, G = nc.vector, nc.gpsimd
                if eng == "v":
                    e_tt = e_sq = e_xb = e_gb = e_gm = e_gp = V
                elif eng == "g":
                    e_tt = e_sq = e_xb = e_gb = e_gm = e_gp = G
                else:
                    e_tt = e_sq = e_xb = e_gb = e_gm = e_gp = (G if EPI_POOL else V)
                zs = saved[b]
                zv = zs[:, lo:hi].rearrange("p a (d e) -> p a d e", d=4)
                tag = f"_{lo}_{w}"
                rinv = epi.tile([128, w, 4], F, tag="rinv" + tag)
                nc.vector.reciprocal(rinv[:], zs[:, lo:hi, 32:132:33])
                nc.vector.tensor_tensor(rinv[:], rinv[:], tlam[:, lo:hi], AL.mult)
                yield
                tt = epi.tile([128, w, 4, 32], F, tag="tt" + tag)
                e_tt.tensor_tensor(tt[:], zv[:, :, :, 0:32],
                                   rinv[:].broadcast_to([128, w, 4, 32]), AL.mult)
                yield
                # u_m = t_{2m} - t_{2m+1}: even/odd d via (m, e*c) split
                t4 = tt[:].rearrange("p a (m e) c -> p a m (e c)", m=2)
                uu = epi.tile([128, w, 2, 32], F, tag="uu" + tag)
                nc.vector.tensor_tensor(uu[:], t4[:, :, :, 0:32], t4[:, :, :, 32:64],
                                        AL.subtract)
                yield
                st = epi.tile([128, w, 2, 8], F, tag="st" + tag)  # stats slots
                nc.vector.tensor_reduce(st[:, :, :, 0:1], uu[:], AX.X, AL.add)
                sq = epi.tile([128, w, 2, 32], F, tag="sq" + tag)
                e_sq.tensor_tensor(sq[:], uu[:], uu[:], AL.mult)
                yield
                nc.vector.tensor_reduce(st[:, :, :, 1:2], sq[:], AX.X, AL.add)
                # mu = sum/32 ; e2 = sumsq/32 ; var = e2 - mu^2 (+eps)
                nc.vector.tensor_scalar(st[:, :, :, 2:3], st[:, :, :, 0:1], 1.0 / 32, None, AL.mult)
                nc.vector.tensor_scalar(st[:, :, :, 3:4], st[:, :, :, 1:2], 1.0 / 32, None, AL.mult)
                yield
                nc.vector.tensor_tensor(st[:, :, :, 4:5], st[:, :, :, 2:3], st[:, :, :, 2:3], AL.mult)
                nc.vector.tensor_tensor(st[:, :, :, 5:6], st[:, :, :, 3:4], st[:, :, :, 4:5], AL.subtract)
                nc.vector.tensor_scalar(st[:, :, :, 6:7], st[:, :, :, 5:6], LN_EPS, None, AL.add)
                # rs = (var+eps)^-0.5 on gpsimd
                nc.gpsimd.tensor_tensor(st[:, :, :, 7:8], st[:, :, :, 6:7],
                                        nhalf[:].broadcast_to([128, w, 2, 1]),
                                        AL.pow)
                yield
                xb = epi.tile([128, w, 2, 32], F, tag="xb" + tag)
                e_xb.tensor_tensor(xb[:], uu[:],
                                   st[:, :, :, 2:3].broadcast_to([128, w, 2, 32]), AL.subtract)
                e_xb.tensor_tensor(xb[:], xb[:],
                                   st[:, :, :, 7:8].broadcast_to([128, w, 2, 32]), AL.mult)
                yield
                e_gb.tensor_tensor(xb[:], xb[:], tgam[:, lo:hi], AL.mult)
                e_gb.tensor_tensor(xb[:], xb[:], tbet[:, lo:hi], AL.add)
                yield
                tgv = tg[:, 4 * b + lo:4 * b + hi, :].rearrange("p a (m c) -> p a m c", m=2)
                gm = epi.tile([128, w, 2, 32], F, tag="gm" + tag)
                e_gm.tensor_tensor(gm[:], xb[:], tgv, AL.mult)
                yield
                xv = xb[:].rearrange("p (P k) m c -> p P (k m c)", P=w // 2)
                mv = gm[:].rearrange("p (P k) m c -> p P (k m c)", P=w // 2)
                gp = gatedp[:, lo // 2:hi // 2]
                for k in range(2):
                    e_gp.tensor_tensor(gp[:, :, 64 * k:64 * k + 64],
                                       xv[:, :, 64 * k:64 * k + 64],
                                       mv[:, :, 64 * k:64 * k + 64], AL.add)
                yield

            def epilogue(b):
                gatedp = epi.tile([128, 2, 128], BF, tag="gatedp")
                if b < NB - 1:
                    for _ in epi_chain(b, 0, 4, "m", gatedp):
                        yield
                else:
                    ca = epi_chain(b, 0, 2, "v", gatedp)
                    cb = epi_chain(b, 2, 4, "g", gatedp)
                    while True:
                        ra = next(ca, "END")
                        rb = next(cb, "END")
                        if ra == "END" and rb == "END":
                            break
                        yield
                gT = []
                if b < NB - 1:
                    for P in range(2):
                        g_t = epi.tile([128, 128], BF, tag=f"gT{P}", name=f"gT{P}")
                        nc.sync.dma_start(out=g_t[:], in_=gatedp[:, P, :], transpose=True)
                        gT.append(g_t)
                else:
                    # last block: PE transpose (short latency) via a dots slot
                    tp = dp.tile([128, 1024], F, tag="dots")
                    for P in range(2):
                        nc.tensor.matmul(tp[:, 64 * P:64 * P + 64].bitcast(BF),
                                         gatedp[:, P, :], tident[:],
                                         is_transpose=True,
                                         start=True, stop=True,
                                         tile_position=(0, 0),
                                         skip_group_check=True)
                    for P in range(2):
                        g_t = epi.tile([128, 128], BF, tag=f"gT{P}", name=f"gT{P}")
                        nc.vector.tensor_copy(g_t[:], tp[:, 64 * P:64 * P + 64].bitcast(BF))
                        gT.append(g_t)
                yield
                yp = dp.tile([128, 1024], F, tag="dots")
                for qs in range(4):
                    P, k = qs >> 1, qs & 1
                    for oh in range(2):
                        nc.tensor.matmul(
                            yp[:, 512 * k + 256 * oh + 128 * P:512 * k + 256 * oh + 128 * P + 128],
                            twout[64 * k:64 * k + 64, oh, :],
                            gT[P][64 * k:64 * k + 64, :],
                            start=True, stop=True,
                            tile_position=(64 * k, 0),
                            skip_group_check=True)
                yield
                ys = epi.tile([128, 1024], F, tag="ys")
                ysv = ys[:].rearrange("p (k oh P q) -> p k oh P q", k=2, oh=2, P=2)
                for k in range(2):
                    nc.vector.tensor_copy(ys[:, 512 * k:512 * k + 512],
                                          yp[:, 512 * k:512 * k + 512])
                for oh in range(2):
                    for P in range(2):
                        nc.sync.dma_start(
                            out=yT[128 * oh:128 * oh + 128,
                                   512 * b + 256 * P:512 * b + 256 * P + 256],
                            in_=ysv[:, :, oh, P, :])
                yield

            # ---------------- schedule ----------------
            proj_qk(kTp, twkp, 0)
            proj_qk(qTp, twqp, 0)
            proj_vg(0)

            pulls0 = {
                2: [lambda: proj_qk(kTp, twkp, 1)],
                3: [lambda: proj_vg(1)],
                6: [lambda: proj_qk(kTp, twkp, 2)],
                7: [lambda: proj_vg(2)],
                9: [lambda: proj_qk(qTp, twqp, 1)],
                10: [lambda: proj_qk(kTp, twkp, 3)],
                11: [lambda: proj_vg(3)],
                13: [lambda: proj_qk(qTp, twqp, 2)],
                14: [lambda: proj_qk(qTp, twqp, 3)],
            }
            attention(0, pulls=pulls0)
            for b in range(1, NB):
                attention(b, interleave=epilogue(b - 1))
            for _ in epilogue(NB - 1):
                pass

    nc.finalize()
    return nc


def _prep_core_inputs(inputs, bi, hg, lam):
    scale = DH ** -0.5
    li = np.float32(1.0 - LAMBDA_INIT)
    x = np.asarray(inputs["x"], np.float32)
    Wq = np.asarray(inputs["Wq"], np.float32)
    Wkv = np.asarray(inputs["Wkv"], np.float32)
    Wout = np.asarray(inputs["Wout"], np.float32)
    Wg = np.asarray(inputs["Wg"], np.float32)
    bg = np.asarray(inputs["bg"], np.float32)
    g_ = np.asarray(inputs["ln_gamma"], np.float32)
    b_ = np.asarray(inputs["ln_beta"], np.float32)

    c0 = 64 * hg
    wq_c = Wq[:, c0:c0 + 64] * scale
    wk_c = Wkv[:, c0:c0 + 64]
    wv_c = Wkv[:, 256 + c0:256 + c0 + 64]
    wg_c = Wg[:, c0:c0 + 64]
    wout_c = Wout[c0:c0 + 64, :]

    wqp = np.zeros((256, 128), np.float32)
    wkp = np.zeros((256, 128), np.float32)
    for d in range(4):
        wqp[:, 32 * d:32 * d + 16] = wq_c[:, 16 * d:16 * d + 16]
        wkp[:, 32 * d:32 * d + 16] = wk_c[:, 16 * d:16 * d + 16]

    wvg = np.zeros((256, 256), np.float32)
    wvg[:, 0:64] = wv_c
    wvg[:, 64:128] = wg_c

    bgrow = np.zeros((1, 256), np.float32)
    bgrow[0, 64:128] = bg[c0:c0 + 64]

    wout2 = np.zeros((128, 2, 128), np.float32)
    for k in range(2):
        for oh in range(2):
            wout2[64 * k:64 * k + 64, oh, :] = wout_c[:, 128 * oh:128 * oh + 128]

    lamq = np.tile(np.array([1.0, lam, 1.0, lam], np.float32), 4)[None, :].repeat(128, 0)
    gam1 = np.tile(g_[0:32] * li * 0.5, 8)[None, :].repeat(128, 0)
    bet1 = np.tile(b_[0:32] * li * 0.5, 8)[None, :].repeat(128, 0)

    return {
        "xT": np.ascontiguousarray(x[bi].T),
        "wqp": wqp, "wkp": wkp, "wvg": wvg,
        "bgrow": bgrow,
        "onesrow": np.ones((1, 128), np.float32),
        "ident": np.eye(128, dtype=np.float32).astype(ml_dtypes.bfloat16),
        "wout2": wout2.astype(ml_dtypes.bfloat16),
        "lamq": np.ascontiguousarray(lamq),
        "gamq": np.ascontiguousarray(gam1),
        "betq": np.ascontiguousarray(bet1),
    }


def kernel(**inputs) -> np.ndarray:
    lq1 = np.asarray(inputs["lq1"], np.float64)
    lk1 = np.asarray(inputs["lk1"], np.float64)
    lq2 = np.asarray(inputs["lq2"], np.float64)
    lk2 = np.asarray(inputs["lk2"], np.float64)
    lam = float(np.exp(np.sum(lq1 * lk1)) - np.exp(np.sum(lq2 * lk2)) + LAMBDA_INIT)
    bout = np.asarray(inputs["bout"], np.float32)

    if "nc" not in _cached:
        _cached["nc"] = build_kernel()
    nc = _cached["nc"]

    in_maps = []
    for c in range(NC):
        bi, hg = c // 4, c % 4
        in_maps.append(_prep_core_inputs(inputs, bi, hg, lam))

    trace = bool(int(os.environ.get("BASS_KERNEL_TRACE", "0")))
    res = run_bass_kernel_spmd(nc, in_maps, list(range(NC)), trace=trace)
    _cached["exec_time_ns"] = res.exec_time_ns
    _cached["trace"] = res.instructions_and_trace
    _cached["res"] = res
    out = np.zeros((B, N, DIM), np.float32)
    for c in range(NC):
        bi = c // 4
        out[bi] += res.results[c]["yT"].T
    out += bout
    return out
